# revision 14
# baseline (speedup 1.0000x reference)
"""Trainium2 Bass kernel for nn_CrossLayer (DCN-style cross stack).

Reference semantics (B=16384, D=1024, L=8):
    out_0 = x
    s_i = einsum('bd,d->b', out_i, W[i])
    out_{i+1} = x * s_i[:, None] + b[i] + x

Algebraic collapse: out_{i+1} = x * rho_{i+1} + b[i] with
    rho_1 = u_0 + 1,   rho_{l+1} = rho_l * u_l + c_l
    u_l[r] = <x[r, :], W[l]>          (U = x @ W.T, [B, L])
    c_l = <b[l-1], W[l]> + 1          (weights-only scalars)
    out = x * rho_8[:, None] + b[L-1]

HBM-bound, so the whole data path is fp16 (median rel err ~1e-3 vs the
2e-2 gate): 4 MiB in + 4 MiB out per core. Per 128-row slot: 8 PE chunk
transposes -> one ACT PSUM->SBUF eviction (fp32-bitcast view halves the
element count; int32 views get mangled by the ACT float datapath) ->
W-stationary fp16 matmuls (U^T) -> DVE cast -> tiny PE transpose ->
DVE scan (fp32 out; 16-bit scan output loses internal precision) ->
out = x*rho + b7 split as ACT mul (per-partition scale) or DVE
tensor_scalar (487ns) + DVE tensor_tensor add (2x packed, 601ns).
scalar_tensor_tensor measured 1x on HW (no fast uops), hence the split.

Layout: group-major, partition p <-> row 128*g + p (2KB descriptors).
Blocks of groups: [1, 2,2,2,2,2,2,2, 1] - single-slot first block
shortens the pipe-fill chain to the first DVE op, single-slot last
block plus a split final TT shortens the drain tail. ident/wt ride the
scalar HWDGE ring first (the first transpose is ident-gated); x blocks
alternate the two HWDGE rings; y slots cycle gpsimd/sync/scalar with
the final halves on the HWDGE rings.

Sharding: data-parallel over batch; 8 cores x 2048 rows. Tiny (L, D)
weights replicated.
"""

import numpy as np

import concourse.bacc as bacc
import concourse.tile as tile
from concourse import mybir
from concourse.bass_utils import run_bass_kernel_spmd

N_CORES = 8
B, D, L = 16384, 1024, 8
RPC = B // N_CORES          # rows per core (2048)
NG = RPC // 128             # 128-row groups per core (16)
NCH = D // 128              # 128-wide d chunks (8)
# blocks as group-runs: single-slot head/tail, 2-slot middle
BLOCKS = [1] + [2] * 7 + [1]

LAST_RESULTS = None


def _build(cvals):
    """Trace + compile the per-core program. cvals = [c_1..c_{L-1}]."""
    nc = bacc.Bacc("TRN2", target_bir_lowering=False, debug=False)
    f32 = mybir.dt.float32
    f16 = mybir.dt.float16

    x_d = nc.dram_tensor("x", [RPC, D], f16, kind="ExternalInput")
    wt_d = nc.dram_tensor("wt", [128, NCH * L], f16, kind="ExternalInput")
    b7_d = nc.dram_tensor("b7r", [128, D], f16, kind="ExternalInput")
    id_d = nc.dram_tensor("ident", [128, 128], f16, kind="ExternalInput")
    y_d = nc.dram_tensor("y", [RPC, D], f16, kind="ExternalOutput")

    # group views: partition p <-> row 128*g + p (2KB descriptors)
    x_g = x_d.ap().rearrange("(g p) d -> g p d", p=128)
    y_g = y_d.ap().rearrange("(g p) d -> g p d", p=128)

    with tile.TileContext(nc) as tc:
        with (
            tc.tile_pool(name="const", bufs=1) as cpool,
            tc.tile_pool(name="xp", bufs=len(BLOCKS)) as xpool,
            tc.tile_pool(name="xtp", bufs=3) as xtpool,
            tc.tile_pool(name="yp", bufs=6) as ypool,
            tc.tile_pool(name="tp", bufs=4) as tpool,
            tc.tile_pool(name="small", bufs=8) as spool,
            tc.tile_pool(name="pst", bufs=3, space="PSUM") as pst,
            tc.tile_pool(name="psu", bufs=2, space="PSUM") as psu,
            tc.tile_pool(name="psr", bufs=2, space="PSUM") as psr,
        ):
            # --- ident/wt ride the scalar HWDGE ring FIRST (~0.7us): the
            # first transpose is ident-gated. b7 (needed later) on gpsimd.
            ident = cpool.tile([128, 128], f16)
            nc.scalar.dma_start(out=ident[:], in_=id_d[:, :])
            wt_sb = cpool.tile([128, NCH, L], f16)
            nc.scalar.dma_start(out=wt_sb[:], in_=wt_d.ap().rearrange("p (c l) -> p c l", l=L))
            b7_sb = cpool.tile([128, D], f16)
            nc.gpsimd.dma_start(out=b7_sb[:], in_=b7_d[:, :])

            # --- all x DMAs up front, one per slot (subtile deps let each
            # slot's transposes start on its own arrival), blocks alternating
            # the two HWDGE rings ---
            xbs = []
            g0 = 0
            for bi, ns in enumerate(BLOCKS):
                xb = xpool.tile([128, ns * D], f16, tag=f"xb{ns}")
                eng = nc.sync if bi % 2 == 0 else nc.scalar
                for s in range(ns):
                    eng.dma_start(out=xb[:, s * D : (s + 1) * D], in_=x_g[g0 + s])
                xbs.append((xb, g0, ns))
                g0 += ns

            # scan constants: cc[:, 0] = 1 (folds the +1 of rho_1), cc[:, l] = c_l
            cc_sb = cpool.tile([128, L], f32)
            nc.gpsimd.memset(cc_sb[:, 0:1], 1.0)
            for l in range(1, L):
                nc.gpsimd.memset(cc_sb[:, l : l + 1], cvals[l - 1])

            n_slots = sum(BLOCKS)
            si = 0          # global slot index
            for bi, (xb, g0, ns) in enumerate(xbs):
                W_ = ns * 128   # moving width for this block
                xb_c = xb[:].rearrange("p (r c d) -> p r c d", r=ns, c=NCH)
                xb_f = xb[:].rearrange("p (r d) -> p r d", r=ns)
                last_blk = bi == len(BLOCKS) - 1

                # transpose chunks -> xT [128d, c, W]; col = s*128 + p
                xT = xtpool.tile([128, NCH, W_], f16, tag=f"xT{ns}")
                for s in range(ns):
                    off = 128 * s
                    pt = pst.tile([128, NCH, 128], f16, tag="pst")
                    for c in range(NCH):
                        nc.tensor.transpose(
                            pt[:, c, :], xb_c[:, s, c, :], ident[:]
                        )
                    # fp32 view halves the ACT element count (same bytes)
                    nc.scalar.copy(
                        xT[:, :, off : off + 128].bitcast(f32), pt[:].bitcast(f32)
                    )

                # U^T for the block: [L, W] = sum_c WT_c.T @ xT_c
                # (fixed-size PSUM tag; sliced for single-slot blocks)
                ps_u = psu.tile([L, 256], f32, tag="psu")
                for c in range(NCH):
                    nc.tensor.matmul(
                        ps_u[:, :W_], wt_sb[:, c, :], xT[:, c, :],
                        start=(c == 0), stop=(c == NCH - 1),
                    )
                ut = spool.tile([L, W_], f16, tag=f"ut{ns}")
                nc.vector.tensor_copy(ut[:], ps_u[:, :W_])

                yt = ypool.tile([128, ns, D], f16, tag=f"yt{ns}")
                for s in range(ns):
                    off = 128 * s
                    # U slot back to row-partition orientation: [128, L]
                    pr = psr.tile([128, L], f16, tag="psr")
                    nc.tensor.transpose(
                        pr[:], ut[:, off : off + 128], ident[0:L, 0:L]
                    )
                    # rho chain: rho_{l+1} = rho_l*u_l + c_l, rho_0 = c_0 = 1
                    # (fp32 out: a 16-bit scan output drops internal precision)
                    scano = spool.tile([128, L], f32, tag="scan")
                    nc.vector.tensor_tensor_scan(
                        scano[:], pr[:], cc_sb[:], 1.0,
                        mybir.AluOpType.mult, mybir.AluOpType.add,
                    )
                    # out = x * rho + b7, multiply engine-split across slots
                    tmp = tpool.tile([128, D], f16, tag="tmp")
                    on_act = (si % 2 == 0) and not last_blk
                    if on_act:
                        # multiply on ACT: per-partition scale AP
                        nc.scalar.mul(tmp[:], xb_f[:, s, :], scano[:, L - 1 : L])
                    else:
                        # multiply on DVE tensor_scalar (fp32 scalar per ISA)
                        nc.vector.tensor_scalar(
                            tmp[:], xb_f[:, s, :], scano[:, L - 1 : L], None,
                            mybir.AluOpType.mult,
                        )
                    if last_blk:
                        # split the final add so the drain starts sooner;
                        # halves go out on the two idle HWDGE rings
                        for h, eng in ((0, nc.sync), (1, nc.scalar)):
                            hd = D // 2
                            nc.vector.tensor_tensor(
                                yt[:, s, h * hd : (h + 1) * hd],
                                tmp[:, h * hd : (h + 1) * hd],
                                b7_sb[:, h * hd : (h + 1) * hd],
                                mybir.AluOpType.add,
                            )
                            eng.dma_start(
                                out=y_g[g0 + s][:, h * hd : (h + 1) * hd],
                                in_=yt[:, s, h * hd : (h + 1) * hd],
                            )
                    else:
                        # bias add on DVE tensor_tensor (2x packed fp16),
                        # then drain this slot; y engines rotate over the
                        # three rings
                        nc.vector.tensor_tensor(
                            yt[:, s, :], tmp[:], b7_sb[:], mybir.AluOpType.add
                        )
                        eng = (nc.gpsimd, nc.sync, nc.scalar)[si % 3]
                        eng.dma_start(out=y_g[g0 + s], in_=yt[:, s, :])
                    si += 1

    nc.compile()
    return nc


def kernel(x, W, b):
    global LAST_RESULTS
    x = np.ascontiguousarray(np.asarray(x), dtype=np.float32)
    W = np.ascontiguousarray(np.asarray(W), dtype=np.float32)
    b = np.ascontiguousarray(np.asarray(b), dtype=np.float32)
    assert x.shape == (B, D) and W.shape == (L, D) and b.shape == (L, D)

    cvals = [float(np.dot(b[l - 1].astype(np.float64), W[l].astype(np.float64)) + 1.0)
             for l in range(1, L)]
    x16 = x.astype(np.float16)
    W16 = W.astype(np.float16)
    wt = W16.T.reshape(NCH, 128, L).transpose(1, 0, 2).reshape(128, NCH * L)
    wt = np.ascontiguousarray(wt, dtype=np.float16)
    b7r = np.ascontiguousarray(
        np.broadcast_to(b[L - 1].astype(np.float16), (128, D)), dtype=np.float16
    )
    ident = np.eye(128, dtype=np.float16)

    nc = _build(cvals)

    shards = [x16[i * RPC : (i + 1) * RPC] for i in range(N_CORES)]
    in_maps = [{"x": s, "wt": wt, "b7r": b7r, "ident": ident} for s in shards]
    res = run_bass_kernel_spmd(nc, in_maps, core_ids=list(range(N_CORES)))
    LAST_RESULTS = res
    out = np.concatenate([res.results[i]["y"] for i in range(N_CORES)], axis=0)
    return out.astype(np.float32)


# revision 15
# speedup vs baseline: 1.1268x; 1.1268x over previous
"""Trainium2 Bass kernel for nn_CrossLayer (DCN-style cross stack).

Reference semantics (B=16384, D=1024, L=8):
    out_0 = x
    s_i = einsum('bd,d->b', out_i, W[i])
    out_{i+1} = x * s_i[:, None] + b[i] + x

Algebraic collapse: out_{i+1} = x * rho_{i+1} + b[i] with
    rho_1 = u_0 + 1,   rho_{l+1} = rho_l * u_l + c_l
    u_l[r] = <x[r, :], W[l]>          (U = x @ W.T, [B, L])
    c_l = <b[l-1], W[l]> + 1          (weights-only scalars)
    out = x * rho_8[:, None] + b[L-1]

HBM-bound, so the whole data path is fp16 (median rel err ~1e-3 vs the
2e-2 gate): 4 MiB in + 4 MiB out per core. Per 128-row slot: 8 PE chunk
transposes -> ACT PSUM->SBUF eviction (fp32-bitcast view halves the
element count; int32 views get mangled by the ACT float datapath) ->
W-stationary fp16 matmuls (U^T) -> DVE cast -> tiny PE transpose ->
DVE scan (fp32 out; a 16-bit scan output loses internal precision) ->
out = x*rho + b7 with the multiply split across engines per slot (ACT
per-partition-scale mul / DVE tensor_scalar) and the add on DVE
tensor_tensor (2x packed). scalar_tensor_tensor measured 1x on HW.

Layout: 256-row blocks, partition p <-> rows 2p/2p+1 (4KB descriptors).
ident/wt ride the scalar HWDGE ring first (the first transpose is
ident-gated; the gpsimd SWDGE ring delivered it ~5us late); x blocks
alternate the two HWDGE rings; block 0 lands as two half-DMAs so
compute starts early; y blocks ride gpsimd (SWDGE, independent ring);
the final block drains as two per-slot halves on the two HWDGE rings.

Sharding: data-parallel over batch; 8 cores x 2048 rows. Tiny (L, D)
weights replicated.
"""

import numpy as np

import concourse.bacc as bacc
import concourse.tile as tile
from concourse import mybir
from concourse.bass_utils import run_bass_kernel_spmd

N_CORES = 8
B, D, L = 16384, 1024, 8
RPC = B // N_CORES          # rows per core (2048)
NB = RPC // 256             # 256-row blocks per core (8)
NCH = D // 128              # 128-wide d chunks (8)

LAST_RESULTS = None


def _build(cvals):
    """Trace + compile the per-core program. cvals = [c_1..c_{L-1}]."""
    nc = bacc.Bacc("TRN2", target_bir_lowering=False, debug=False)
    f32 = mybir.dt.float32
    f16 = mybir.dt.float16

    x_d = nc.dram_tensor("x", [RPC, D], f16, kind="ExternalInput")
    wt_d = nc.dram_tensor("wt", [128, NCH * L], f16, kind="ExternalInput")
    b7_d = nc.dram_tensor("b7r", [128, D], f16, kind="ExternalInput")
    id_d = nc.dram_tensor("ident", [128, 128], f16, kind="ExternalInput")
    y_d = nc.dram_tensor("y", [RPC, D], f16, kind="ExternalOutput")

    # block views: partition p <-> rows 2p, 2p+1 of the block (4KB descr.)
    x_blk = x_d.ap().rearrange("(t p r) d -> t p (r d)", p=128, r=2)
    y_blk = y_d.ap().rearrange("(t p r) d -> t p (r d)", p=128, r=2)

    with tile.TileContext(nc) as tc:
        with (
            tc.tile_pool(name="const", bufs=1) as cpool,
            tc.tile_pool(name="xp", bufs=8) as xpool,
            tc.tile_pool(name="xtp", bufs=3) as xtpool,
            tc.tile_pool(name="yp", bufs=4) as ypool,
            tc.tile_pool(name="tp", bufs=4) as tpool,
            tc.tile_pool(name="small", bufs=8) as spool,
            tc.tile_pool(name="pst", bufs=2, space="PSUM") as pst,
            tc.tile_pool(name="psu", bufs=2, space="PSUM") as psu,
            tc.tile_pool(name="psr", bufs=2, space="PSUM") as psr,
        ):
            # --- ident/wt ride the scalar HWDGE ring FIRST (~0.7us): the
            # first transpose is ident-gated. b7 (needed later) on gpsimd.
            ident = cpool.tile([128, 128], f16)
            nc.scalar.dma_start(out=ident[:], in_=id_d[:, :])
            wt_sb = cpool.tile([128, NCH, L], f16)
            nc.scalar.dma_start(out=wt_sb[:], in_=wt_d.ap().rearrange("p (c l) -> p c l", l=L))
            b7_sb = cpool.tile([128, D], f16)
            nc.gpsimd.dma_start(out=b7_sb[:], in_=b7_d[:, :])

            # --- all x input DMAs up front, alternating the two HWDGE rings.
            # Block 0 lands as two half-DMAs so slot-0 compute starts early.
            xbs = []
            for i in range(NB):
                xb = xpool.tile([128, 2 * D], f16, tag="xb")
                eng = nc.sync if i % 2 == 0 else nc.scalar
                if i == 0:
                    eng.dma_start(out=xb[:, 0:D], in_=x_blk[i][:, 0:D])
                    eng.dma_start(out=xb[:, D : 2 * D], in_=x_blk[i][:, D : 2 * D])
                else:
                    eng.dma_start(out=xb[:], in_=x_blk[i])
                xbs.append(xb)

            # scan constants: cc[:, 0] = 1 (folds the +1 of rho_1), cc[:, l] = c_l
            cc_sb = cpool.tile([128, L], f32)
            nc.gpsimd.memset(cc_sb[:, 0:1], 1.0)
            for l in range(1, L):
                nc.gpsimd.memset(cc_sb[:, l : l + 1], cvals[l - 1])

            for i in range(NB):
                xb = xbs[i]
                # [p, slot, chunk, 128] and [p, slot, 1024] views
                xb_c = xb[:].rearrange("p (r c d) -> p r c d", r=2, c=NCH)
                xb_f = xb[:].rearrange("p (r d) -> p r d", r=2)

                # transpose chunks -> xT [128d, c, 256]; col = s*128 + p
                xT = xtpool.tile([128, NCH, 256], f16, tag="xT")
                for s in range(2):
                    off = 128 * s
                    pt = pst.tile([128, NCH, 128], f16, tag="pst")
                    for c in range(NCH):
                        nc.tensor.transpose(
                            pt[:, c, :], xb_c[:, s, c, :], ident[:]
                        )
                    # fp32 view halves the ACT element count (same bytes).
                    # NOT int32: the ACT float datapath mangles int bits
                    # (median err jumped 1e-3 -> 7e-3); fp32 Copy is the
                    # standard bit-exact PSUM-eviction path.
                    nc.scalar.copy(
                        xT[:, :, off : off + 128].bitcast(f32), pt[:].bitcast(f32)
                    )

                # U^T for the block: [L, 256] = sum_c WT_c.T @ xT_c
                ps_u = psu.tile([L, 256], f32, tag="psu")
                for c in range(NCH):
                    nc.tensor.matmul(
                        ps_u[:], wt_sb[:, c, :], xT[:, c, :],
                        start=(c == 0), stop=(c == NCH - 1),
                    )
                ut = spool.tile([L, 256], f16, tag="ut")
                nc.vector.tensor_copy(ut[:], ps_u[:])

                yt = ypool.tile([128, 2, D], f16, tag="yt")
                last = i == NB - 1
                for s in range(2):
                    off = 128 * s
                    # U slot back to row-partition orientation: [128, L]
                    pr = psr.tile([128, L], f16, tag="psr")
                    nc.tensor.transpose(
                        pr[:], ut[:, off : off + 128], ident[0:L, 0:L]
                    )
                    # rho chain: rho_{l+1} = rho_l*u_l + c_l, rho_0 = c_0 = 1
                    # (fp32 out: a 16-bit scan output drops internal precision)
                    scano = spool.tile([128, L], f32, tag="scan")
                    nc.vector.tensor_tensor_scan(
                        scano[:], pr[:], cc_sb[:], 1.0,
                        mybir.AluOpType.mult, mybir.AluOpType.add,
                    )
                    # out = x * rho + b7, engine-split per slot
                    tmp = tpool.tile([128, D], f16, tag="tmp")
                    if s == 0:
                        # multiply on ACT: per-partition scale AP
                        nc.scalar.mul(tmp[:], xb_f[:, s, :], scano[:, L - 1 : L])
                    else:
                        # multiply on DVE tensor_scalar (fp32 scalar per ISA)
                        nc.vector.tensor_scalar(
                            tmp[:], xb_f[:, s, :], scano[:, L - 1 : L], None,
                            mybir.AluOpType.mult,
                        )
                    # bias add on DVE tensor_tensor (2x packed fp16)
                    nc.vector.tensor_tensor(
                        yt[:, s, :], tmp[:], b7_sb[:], mybir.AluOpType.add
                    )
                    if last:
                        # final block drains as two per-slot halves on the
                        # two HWDGE queues (inputs done; lowest completion
                        # latency)
                        eng = nc.scalar if s == 0 else nc.sync
                        eng.dma_start(
                            out=y_blk[i][:, D * s : D * (s + 1)], in_=yt[:, s, :]
                        )
                if not last:
                    # outputs ride the gpsimd (SWDGE) ring - independent of
                    # the input queues, so they never FIFO-stall behind x
                    nc.gpsimd.dma_start(out=y_blk[i], in_=yt[:])

    nc.compile()
    return nc


def kernel(x, W, b):
    global LAST_RESULTS
    x = np.ascontiguousarray(np.asarray(x), dtype=np.float32)
    W = np.ascontiguousarray(np.asarray(W), dtype=np.float32)
    b = np.ascontiguousarray(np.asarray(b), dtype=np.float32)
    assert x.shape == (B, D) and W.shape == (L, D) and b.shape == (L, D)

    cvals = [float(np.dot(b[l - 1].astype(np.float64), W[l].astype(np.float64)) + 1.0)
             for l in range(1, L)]
    x16 = x.astype(np.float16)
    W16 = W.astype(np.float16)
    wt = W16.T.reshape(NCH, 128, L).transpose(1, 0, 2).reshape(128, NCH * L)
    wt = np.ascontiguousarray(wt, dtype=np.float16)
    b7r = np.ascontiguousarray(
        np.broadcast_to(b[L - 1].astype(np.float16), (128, D)), dtype=np.float16
    )
    ident = np.eye(128, dtype=np.float16)

    nc = _build(cvals)

    shards = [x16[i * RPC : (i + 1) * RPC] for i in range(N_CORES)]
    in_maps = [{"x": s, "wt": wt, "b7r": b7r, "ident": ident} for s in shards]
    res = run_bass_kernel_spmd(nc, in_maps, core_ids=list(range(N_CORES)))
    LAST_RESULTS = res
    out = np.concatenate([res.results[i]["y"] for i in range(N_CORES)], axis=0)
    return out.astype(np.float32)


# revision 16
# speedup vs baseline: 1.1291x; 1.0020x over previous
"""Trainium2 Bass kernel for nn_CrossLayer (DCN-style cross stack).

Reference semantics (B=16384, D=1024, L=8):
    out_0 = x
    s_i = einsum('bd,d->b', out_i, W[i])
    out_{i+1} = x * s_i[:, None] + b[i] + x

Algebraic collapse: out_{i+1} = x * rho_{i+1} + b[i] with
    rho_1 = u_0 + 1,   rho_{l+1} = rho_l * u_l + c_l
    u_l[r] = <x[r, :], W[l]>          (U = x @ W.T, [B, L])
    c_l = <b[l-1], W[l]> + 1          (weights-only scalars)
    out = x * rho_8[:, None] + b[L-1]

HBM-bound, so the whole data path is fp16 (median rel err ~1e-3 vs the
2e-2 gate): 4 MiB in + 4 MiB out per core. Per 128-row slot: 8 PE chunk
transposes -> ACT PSUM->SBUF eviction (fp32-bitcast view halves the
element count; int32 views get mangled by the ACT float datapath) ->
W-stationary fp16 matmuls (U^T) -> DVE cast -> tiny PE transpose ->
DVE scan (fp32 out; a 16-bit scan output loses internal precision) ->
out = x*rho + b7 with the multiply split across engines per slot (ACT
per-partition-scale mul / DVE tensor_scalar) and the add on DVE
tensor_tensor (2x packed). scalar_tensor_tensor measured 1x on HW.

Layout: 256-row blocks, partition p <-> rows 2p/2p+1 (4KB descriptors).
ident/wt ride the scalar HWDGE ring first (the first transpose is
ident-gated; the gpsimd SWDGE ring delivered it ~5us late); x blocks
alternate the two HWDGE rings; block 0 lands as two half-DMAs so
compute starts early; y blocks ride gpsimd (SWDGE, independent ring);
the final block drains as two per-slot halves on the two HWDGE rings.

Sharding: data-parallel over batch; 8 cores x 2048 rows. Tiny (L, D)
weights replicated.
"""

import numpy as np

import concourse.bacc as bacc
import concourse.tile as tile
from concourse import mybir
from concourse.bass_utils import run_bass_kernel_spmd

N_CORES = 8
B, D, L = 16384, 1024, 8
RPC = B // N_CORES          # rows per core (2048)
NB = RPC // 256             # 256-row blocks per core (8)
NCH = D // 128              # 128-wide d chunks (8)

LAST_RESULTS = None


def _build(cvals):
    """Trace + compile the per-core program. cvals = [c_1..c_{L-1}]."""
    nc = bacc.Bacc("TRN2", target_bir_lowering=False, debug=False)
    f32 = mybir.dt.float32
    f16 = mybir.dt.float16

    x_d = nc.dram_tensor("x", [RPC, D], f16, kind="ExternalInput")
    wt_d = nc.dram_tensor("wt", [128, NCH * L], f16, kind="ExternalInput")
    b7_d = nc.dram_tensor("b7r", [128, D], f16, kind="ExternalInput")
    id_d = nc.dram_tensor("ident", [128, 128], f16, kind="ExternalInput")
    y_d = nc.dram_tensor("y", [RPC, D], f16, kind="ExternalOutput")

    # block views: partition p <-> rows 2p, 2p+1 of the block (4KB descr.)
    x_blk = x_d.ap().rearrange("(t p r) d -> t p (r d)", p=128, r=2)
    y_blk = y_d.ap().rearrange("(t p r) d -> t p (r d)", p=128, r=2)

    with tile.TileContext(nc) as tc:
        with (
            tc.tile_pool(name="const", bufs=1) as cpool,
            tc.tile_pool(name="xp", bufs=8) as xpool,
            tc.tile_pool(name="xtp", bufs=3) as xtpool,
            tc.tile_pool(name="yp", bufs=4) as ypool,
            tc.tile_pool(name="tp", bufs=4) as tpool,
            tc.tile_pool(name="small", bufs=8) as spool,
            tc.tile_pool(name="pst", bufs=2, space="PSUM") as pst,
            tc.tile_pool(name="psu", bufs=2, space="PSUM") as psu,
            tc.tile_pool(name="psr", bufs=2, space="PSUM") as psr,
        ):
            # --- ident/wt ride the sync HWDGE ring FIRST (tiny, ~0.1us of
            # transfer ahead of x0): the first transpose is ident-gated and
            # the gpsimd SWDGE ring delivered it ~5us late, while putting it
            # on the scalar ring delayed x1 and starved the pipeline.
            # b7 (needed later) on gpsimd.
            ident = cpool.tile([128, 128], f16)
            nc.sync.dma_start(out=ident[:], in_=id_d[:, :])
            wt_sb = cpool.tile([128, NCH, L], f16)
            nc.sync.dma_start(out=wt_sb[:], in_=wt_d.ap().rearrange("p (c l) -> p c l", l=L))
            b7_sb = cpool.tile([128, D], f16)
            nc.gpsimd.dma_start(out=b7_sb[:], in_=b7_d[:, :])

            # --- all x input DMAs up front, alternating the two HWDGE rings.
            # Block 0 lands as two half-DMAs so slot-0 compute starts early.
            xbs = []
            for i in range(NB):
                xb = xpool.tile([128, 2 * D], f16, tag="xb")
                eng = nc.sync if i % 2 == 0 else nc.scalar
                if i == 0:
                    eng.dma_start(out=xb[:, 0:D], in_=x_blk[i][:, 0:D])
                    eng.dma_start(out=xb[:, D : 2 * D], in_=x_blk[i][:, D : 2 * D])
                else:
                    eng.dma_start(out=xb[:], in_=x_blk[i])
                xbs.append(xb)

            # scan constants: cc[:, 0] = 1 (folds the +1 of rho_1), cc[:, l] = c_l
            cc_sb = cpool.tile([128, L], f32)
            nc.gpsimd.memset(cc_sb[:, 0:1], 1.0)
            for l in range(1, L):
                nc.gpsimd.memset(cc_sb[:, l : l + 1], cvals[l - 1])

            for i in range(NB):
                xb = xbs[i]
                # [p, slot, chunk, 128] and [p, slot, 1024] views
                xb_c = xb[:].rearrange("p (r c d) -> p r c d", r=2, c=NCH)
                xb_f = xb[:].rearrange("p (r d) -> p r d", r=2)

                # transpose chunks -> xT [128d, c, 256]; col = s*128 + p
                xT = xtpool.tile([128, NCH, 256], f16, tag="xT")
                for s in range(2):
                    off = 128 * s
                    pt = pst.tile([128, NCH, 128], f16, tag="pst")
                    for c in range(NCH):
                        nc.tensor.transpose(
                            pt[:, c, :], xb_c[:, s, c, :], ident[:]
                        )
                    # fp32 view halves the ACT element count (same bytes).
                    # NOT int32: the ACT float datapath mangles int bits
                    # (median err jumped 1e-3 -> 7e-3); fp32 Copy is the
                    # standard bit-exact PSUM-eviction path.
                    nc.scalar.copy(
                        xT[:, :, off : off + 128].bitcast(f32), pt[:].bitcast(f32)
                    )

                # U^T for the block: [L, 256] = sum_c WT_c.T @ xT_c
                ps_u = psu.tile([L, 256], f32, tag="psu")
                for c in range(NCH):
                    nc.tensor.matmul(
                        ps_u[:], wt_sb[:, c, :], xT[:, c, :],
                        start=(c == 0), stop=(c == NCH - 1),
                    )
                ut = spool.tile([L, 256], f16, tag="ut")
                nc.vector.tensor_copy(ut[:], ps_u[:])

                yt = ypool.tile([128, 2, D], f16, tag="yt")
                last = i == NB - 1
                for s in range(2):
                    off = 128 * s
                    # U slot back to row-partition orientation: [128, L]
                    pr = psr.tile([128, L], f16, tag="psr")
                    nc.tensor.transpose(
                        pr[:], ut[:, off : off + 128], ident[0:L, 0:L]
                    )
                    # rho chain: rho_{l+1} = rho_l*u_l + c_l, rho_0 = c_0 = 1
                    # (fp32 out: a 16-bit scan output drops internal precision)
                    scano = spool.tile([128, L], f32, tag="scan")
                    nc.vector.tensor_tensor_scan(
                        scano[:], pr[:], cc_sb[:], 1.0,
                        mybir.AluOpType.mult, mybir.AluOpType.add,
                    )
                    # out = x * rho + b7, engine-split per slot
                    tmp = tpool.tile([128, D], f16, tag="tmp")
                    if s == 0:
                        # multiply on ACT: per-partition scale AP
                        nc.scalar.mul(tmp[:], xb_f[:, s, :], scano[:, L - 1 : L])
                    else:
                        # multiply on DVE tensor_scalar (fp32 scalar per ISA)
                        nc.vector.tensor_scalar(
                            tmp[:], xb_f[:, s, :], scano[:, L - 1 : L], None,
                            mybir.AluOpType.mult,
                        )
                    # bias add on DVE tensor_tensor (2x packed fp16)
                    nc.vector.tensor_tensor(
                        yt[:, s, :], tmp[:], b7_sb[:], mybir.AluOpType.add
                    )
                    if last:
                        # final block drains as two per-slot halves on the
                        # two HWDGE queues (inputs done; lowest completion
                        # latency)
                        eng = nc.scalar if s == 0 else nc.sync
                        eng.dma_start(
                            out=y_blk[i][:, D * s : D * (s + 1)], in_=yt[:, s, :]
                        )
                if not last:
                    # outputs ride the gpsimd (SWDGE) ring - independent of
                    # the input queues, so they never FIFO-stall behind x
                    nc.gpsimd.dma_start(out=y_blk[i], in_=yt[:])

    nc.compile()
    return nc


def kernel(x, W, b):
    global LAST_RESULTS
    x = np.ascontiguousarray(np.asarray(x), dtype=np.float32)
    W = np.ascontiguousarray(np.asarray(W), dtype=np.float32)
    b = np.ascontiguousarray(np.asarray(b), dtype=np.float32)
    assert x.shape == (B, D) and W.shape == (L, D) and b.shape == (L, D)

    cvals = [float(np.dot(b[l - 1].astype(np.float64), W[l].astype(np.float64)) + 1.0)
             for l in range(1, L)]
    x16 = x.astype(np.float16)
    W16 = W.astype(np.float16)
    wt = W16.T.reshape(NCH, 128, L).transpose(1, 0, 2).reshape(128, NCH * L)
    wt = np.ascontiguousarray(wt, dtype=np.float16)
    b7r = np.ascontiguousarray(
        np.broadcast_to(b[L - 1].astype(np.float16), (128, D)), dtype=np.float16
    )
    ident = np.eye(128, dtype=np.float16)

    nc = _build(cvals)

    shards = [x16[i * RPC : (i + 1) * RPC] for i in range(N_CORES)]
    in_maps = [{"x": s, "wt": wt, "b7r": b7r, "ident": ident} for s in shards]
    res = run_bass_kernel_spmd(nc, in_maps, core_ids=list(range(N_CORES)))
    LAST_RESULTS = res
    out = np.concatenate([res.results[i]["y"] for i in range(N_CORES)], axis=0)
    return out.astype(np.float32)


# revision 17
# speedup vs baseline: 1.1920x; 1.0557x over previous
"""Trainium2 Bass kernel for nn_CrossLayer (DCN-style cross stack).

Reference semantics (B=16384, D=1024, L=8):
    out_0 = x
    s_i = einsum('bd,d->b', out_i, W[i])
    out_{i+1} = x * s_i[:, None] + b[i] + x

Algebraic collapse: out_{i+1} = x * rho_{i+1} + b[i] with
    rho_1 = u_0 + 1,   rho_{l+1} = rho_l * u_l + c_l
    u_l[r] = <x[r, :], W[l]>          (U = x @ W.T, [B, L])
    c_l = <b[l-1], W[l]> + 1          (weights-only scalars)
    out = x * rho_8[:, None] + b[L-1]

HBM-bound, so the whole data path is fp16 (median rel err ~1e-3 vs the
2e-2 gate): 4 MiB in + 4 MiB out per core. Per 128-row slot: 8 PE chunk
transposes -> ACT PSUM->SBUF eviction (fp32-bitcast view halves the
element count; int32 views get mangled by the ACT float datapath) ->
W-stationary fp16 matmuls (U^T) -> DVE cast -> tiny PE transpose ->
DVE scan (fp32 out; a 16-bit scan output loses internal precision) ->
out = x*rho + b7 with the multiply split across engines per slot (ACT
per-partition-scale mul / DVE tensor_scalar) and the add on DVE
tensor_tensor (2x packed). scalar_tensor_tensor measured 1x on HW.

Layout: 256-row blocks, partition p <-> rows 2p/2p+1 (4KB descriptors).
ident/wt ride the scalar HWDGE ring first (the first transpose is
ident-gated; the gpsimd SWDGE ring delivered it ~5us late); x blocks
alternate the two HWDGE rings; block 0 lands as two half-DMAs so
compute starts early; y blocks ride gpsimd (SWDGE, independent ring);
the final block drains as two per-slot halves on the two HWDGE rings.

Sharding: data-parallel over batch; 8 cores x 2048 rows. Tiny (L, D)
weights replicated.
"""

import numpy as np

import concourse.bacc as bacc
import concourse.tile as tile
from concourse import mybir
from concourse.bass_utils import run_bass_kernel_spmd

N_CORES = 8
B, D, L = 16384, 1024, 8
RPC = B // N_CORES          # rows per core (2048)
NB = RPC // 256             # 256-row blocks per core (8)
NCH = D // 128              # 128-wide d chunks (8)

LAST_RESULTS = None


def _build(cvals):
    """Trace + compile the per-core program. cvals = [c_1..c_{L-1}]."""
    nc = bacc.Bacc("TRN2", target_bir_lowering=False, debug=False)
    f32 = mybir.dt.float32
    f16 = mybir.dt.float16

    x_d = nc.dram_tensor("x", [RPC, D], f16, kind="ExternalInput")
    wt_d = nc.dram_tensor("wt", [128, NCH * L], f16, kind="ExternalInput")
    b7_d = nc.dram_tensor("b7r", [128, D], f16, kind="ExternalInput")
    id_d = nc.dram_tensor("ident", [128, 128], f16, kind="ExternalInput")
    y_d = nc.dram_tensor("y", [RPC, D], f16, kind="ExternalOutput")

    # block views: partition p <-> rows 2p, 2p+1 of the block (4KB descr.)
    x_blk = x_d.ap().rearrange("(t p r) d -> t p (r d)", p=128, r=2)
    y_blk = y_d.ap().rearrange("(t p r) d -> t p (r d)", p=128, r=2)

    with tile.TileContext(nc) as tc:
        with (
            tc.tile_pool(name="const", bufs=1) as cpool,
            tc.tile_pool(name="xp", bufs=8) as xpool,
            tc.tile_pool(name="xtp", bufs=3) as xtpool,
            tc.tile_pool(name="yp", bufs=4) as ypool,
            tc.tile_pool(name="tp", bufs=4) as tpool,
            tc.tile_pool(name="small", bufs=8) as spool,
            tc.tile_pool(name="pst", bufs=2, space="PSUM") as pst,
            tc.tile_pool(name="psu", bufs=2, space="PSUM") as psu,
            tc.tile_pool(name="psr", bufs=2, space="PSUM") as psr,
        ):
            # --- constants on the gpsimd (SWDGE) ring, ident first (the
            # first transpose is ident-gated). Keeping them off the HWDGE
            # rings leaves both free for the x stream; moving ident to a
            # HWDGE ring measured ~3us WORSE overall (x delivery delayed).
            ident = cpool.tile([128, 128], f16)
            nc.gpsimd.dma_start(out=ident[:], in_=id_d[:, :])
            wt_sb = cpool.tile([128, NCH, L], f16)
            nc.gpsimd.dma_start(out=wt_sb[:], in_=wt_d.ap().rearrange("p (c l) -> p c l", l=L))
            b7_sb = cpool.tile([128, D], f16)
            nc.gpsimd.dma_start(out=b7_sb[:], in_=b7_d[:, :])

            # --- all x input DMAs up front, alternating the two HWDGE rings.
            # Block 0 lands as two half-DMAs so slot-0 compute starts early.
            xbs = []
            for i in range(NB):
                xb = xpool.tile([128, 2 * D], f16, tag="xb")
                eng = nc.sync if i % 2 == 0 else nc.scalar
                if i == 0:
                    eng.dma_start(out=xb[:, 0:D], in_=x_blk[i][:, 0:D])
                    eng.dma_start(out=xb[:, D : 2 * D], in_=x_blk[i][:, D : 2 * D])
                else:
                    eng.dma_start(out=xb[:], in_=x_blk[i])
                xbs.append(xb)

            # scan constants: cc[:, 0] = 1 (folds the +1 of rho_1), cc[:, l] = c_l
            cc_sb = cpool.tile([128, L], f32)
            nc.gpsimd.memset(cc_sb[:, 0:1], 1.0)
            for l in range(1, L):
                nc.gpsimd.memset(cc_sb[:, l : l + 1], cvals[l - 1])

            for i in range(NB):
                xb = xbs[i]
                # [p, slot, chunk, 128] and [p, slot, 1024] views
                xb_c = xb[:].rearrange("p (r c d) -> p r c d", r=2, c=NCH)
                xb_f = xb[:].rearrange("p (r d) -> p r d", r=2)

                # transpose chunks -> xT [128d, c, 256]; col = s*128 + p
                xT = xtpool.tile([128, NCH, 256], f16, tag="xT")
                for s in range(2):
                    off = 128 * s
                    pt = pst.tile([128, NCH, 128], f16, tag="pst")
                    for c in range(NCH):
                        nc.tensor.transpose(
                            pt[:, c, :], xb_c[:, s, c, :], ident[:]
                        )
                    # fp32 view halves the ACT element count (same bytes).
                    # NOT int32: the ACT float datapath mangles int bits
                    # (median err jumped 1e-3 -> 7e-3); fp32 Copy is the
                    # standard bit-exact PSUM-eviction path.
                    nc.scalar.copy(
                        xT[:, :, off : off + 128].bitcast(f32), pt[:].bitcast(f32)
                    )

                # U^T for the block: [L, 256] = sum_c WT_c.T @ xT_c
                ps_u = psu.tile([L, 256], f32, tag="psu")
                for c in range(NCH):
                    nc.tensor.matmul(
                        ps_u[:], wt_sb[:, c, :], xT[:, c, :],
                        start=(c == 0), stop=(c == NCH - 1),
                    )
                ut = spool.tile([L, 256], f16, tag="ut")
                nc.vector.tensor_copy(ut[:], ps_u[:])

                yt = ypool.tile([128, 2, D], f16, tag="yt")
                last = i == NB - 1
                for s in range(2):
                    off = 128 * s
                    # U slot back to row-partition orientation: [128, L]
                    pr = psr.tile([128, L], f16, tag="psr")
                    nc.tensor.transpose(
                        pr[:], ut[:, off : off + 128], ident[0:L, 0:L]
                    )
                    # rho chain: rho_{l+1} = rho_l*u_l + c_l, rho_0 = c_0 = 1
                    # (fp32 out: a 16-bit scan output drops internal precision)
                    scano = spool.tile([128, L], f32, tag="scan")
                    nc.vector.tensor_tensor_scan(
                        scano[:], pr[:], cc_sb[:], 1.0,
                        mybir.AluOpType.mult, mybir.AluOpType.add,
                    )
                    # out = x * rho + b7, engine-split per slot
                    tmp = tpool.tile([128, D], f16, tag="tmp")
                    if s == 0:
                        # multiply on ACT: per-partition scale AP
                        nc.scalar.mul(tmp[:], xb_f[:, s, :], scano[:, L - 1 : L])
                    else:
                        # multiply on DVE tensor_scalar (fp32 scalar per ISA)
                        nc.vector.tensor_scalar(
                            tmp[:], xb_f[:, s, :], scano[:, L - 1 : L], None,
                            mybir.AluOpType.mult,
                        )
                    # bias add on DVE tensor_tensor (2x packed fp16)
                    nc.vector.tensor_tensor(
                        yt[:, s, :], tmp[:], b7_sb[:], mybir.AluOpType.add
                    )
                    if last:
                        # final block drains as two per-slot halves on the
                        # two HWDGE queues (inputs done; lowest completion
                        # latency)
                        eng = nc.scalar if s == 0 else nc.sync
                        eng.dma_start(
                            out=y_blk[i][:, D * s : D * (s + 1)], in_=yt[:, s, :]
                        )
                if not last:
                    # outputs ride the gpsimd (SWDGE) ring - independent of
                    # the input queues, so they never FIFO-stall behind x
                    nc.gpsimd.dma_start(out=y_blk[i], in_=yt[:])

    nc.compile()
    return nc


def kernel(x, W, b):
    global LAST_RESULTS
    x = np.ascontiguousarray(np.asarray(x), dtype=np.float32)
    W = np.ascontiguousarray(np.asarray(W), dtype=np.float32)
    b = np.ascontiguousarray(np.asarray(b), dtype=np.float32)
    assert x.shape == (B, D) and W.shape == (L, D) and b.shape == (L, D)

    cvals = [float(np.dot(b[l - 1].astype(np.float64), W[l].astype(np.float64)) + 1.0)
             for l in range(1, L)]
    x16 = x.astype(np.float16)
    W16 = W.astype(np.float16)
    wt = W16.T.reshape(NCH, 128, L).transpose(1, 0, 2).reshape(128, NCH * L)
    wt = np.ascontiguousarray(wt, dtype=np.float16)
    b7r = np.ascontiguousarray(
        np.broadcast_to(b[L - 1].astype(np.float16), (128, D)), dtype=np.float16
    )
    ident = np.eye(128, dtype=np.float16)

    nc = _build(cvals)

    shards = [x16[i * RPC : (i + 1) * RPC] for i in range(N_CORES)]
    in_maps = [{"x": s, "wt": wt, "b7r": b7r, "ident": ident} for s in shards]
    res = run_bass_kernel_spmd(nc, in_maps, core_ids=list(range(N_CORES)))
    LAST_RESULTS = res
    out = np.concatenate([res.results[i]["y"] for i in range(N_CORES)], axis=0)
    return out.astype(np.float32)
